# revision 21
# baseline (speedup 1.0000x reference)
"""Trainium2 Bass kernel for nn_EpistemicQuantizer (vq_codebook).

Reference forward semantics:
    x_flat  = x.reshape(N, D)
    sims    = l2norm(x_flat) @ l2norm(codebook).T          # (N, V)
    indices = argmax_v sims                                # (N,)
    soft    = softmax((sims + gumbel)/tau)
    term    = soft @ sg(codebook)
    z_q     = codebook[indices] + term - sg(term)          # == codebook[indices] (+~1e-7 fp noise)
    out     = x + sg(z_q - x) + (z_q - sg(z_q))            # == z_q  (+~1e-7 fp noise)

The forward value is exactly (codebook[indices], indices) up to ~1e-7
relative fp noise: term - sg(term) and z_q - sg(z_q) cancel in the
forward pass, so the 256MB gumbel_noise tensor and the softmax never
affect the output.  Also argmax_v cos(x, c_v) == argmax_v (x . c_v /
||c_v||): per-token normalization of x is a positive per-row scale.

Device algorithm, data-parallel over tokens (2048 tokens/core x 8 cores):
  prep: normalize codebook rows (fp32), build fp16 hi/lo split of x and
        cbn, transpose to contraction-major layouts.
  sims via exact-enough fp16 decomposition (x = xh + xl, c = ch + cl):
        sims = xh@ch + [xh;xl]@[cl;ch]     (2 matmuls, error <= ~3e-6;
        measured 0 argmax flips vs fp64 on this problem's data, min
        top1-top2 gap is 9.8e-7 ... 1e-5 for the closest tokens)
  per 128-token tile:
    PE   : sims tile (128, 4096) fp32 in PSUM, 4 chunks of (128, 1024)
    ACT  : drain each PSUM chunk -> sims_sb in SBUF
    DVE  : tensor_tensor_scan(max, max) over (sims_sb half A, half B):
           one pass producing the running max of the pairwise fold; the
           last column is the global max, and max_index of it over the
           scan array is the first folded position p achieving it.
           Candidates v = p and v = p + 2048 are then fetched per tile
           with single-offset indirect-DMA gathers of combined
           [cbn | cb] rows (overlapped with later tiles).
  batched endgame (all 16 tiles at once on DVE):
    exact fp32 rescore d_i = x . cbn[cand_i] (min fold-partner gap here
    is 5.1e-3 vs ~2e-7 rescore noise), winner select by is_ge +
    copy_predicated; z_q taken from the raw halves of the gathered rows
    (no second gather); strided DMA writes z_q and int32 indices.
"""

import os
import sys

import numpy as np

for _p in ("/opt/trn_rl_repo", "/root/.axon_site/_ro/trn_rl_repo"):
    if os.path.isdir(_p) and _p not in sys.path:
        sys.path.insert(0, _p)

from concourse import bacc, bass, mybir  # noqa: E402
from concourse.masks import make_identity  # noqa: E402
from concourse.tile import TileContext  # noqa: E402

P = 128                    # partitions / tokens per tile
D = 64                     # token dim
V = 4096                   # codebook rows
N_CORES = 8
B, T = 8, 2048
NTOK = B * T // N_CORES    # tokens per core = 2048
TILES = NTOK // P          # 16
VH = V // 2                # folded width 2048
CBT = V // P               # 32 codebook tiles

F32 = mybir.dt.float32
F16 = mybir.dt.float16
I32 = mybir.dt.int32
U32 = mybir.dt.uint32
A = mybir.AluOpType
AF = mybir.ActivationFunctionType
NEG_INF = -3.0e38

MM_MODE = "f16x2"  # "f16x2" (2 fp16 matmuls/chunk) or "f32" (1 fp32 = 4 passes)


def build_nc(mm_mode=None):
    mm_mode = MM_MODE if mm_mode is None else mm_mode
    nc = bacc.Bacc("TRN2", target_bir_lowering=False)

    x_d = nc.dram_tensor("x", (NTOK, D), F32, kind="ExternalInput")
    cb_d = nc.dram_tensor("cb", (V, D), F32, kind="ExternalInput")
    zq_d = nc.dram_tensor("zq", (NTOK, D), F32, kind="ExternalOutput")
    idx_d = nc.dram_tensor("idx", (NTOK, 1), I32, kind="ExternalOutput")
    # combined scratch: row v = [cbn[v] (64) | cb[v] (64)] so one gather per
    # candidate fetches both the normalized row (rescore dot) and the raw row
    # (z_q output)
    cbn_d = nc.dram_tensor("cbn", (V, 2 * D), F32)

    with TileContext(nc) as tc:
        with (
            tc.tile_pool(name="const", bufs=1) as constp,
            tc.tile_pool(name="work", bufs=2) as workp,
            tc.tile_pool(name="small", bufs=3) as smallp,
        ):
            # ---------- loads ------------------------------------------
            # x_sb[p, t*D + d] = x[t*P + p, d]
            x_sb = constp.tile([P, TILES * D], F32)
            nc.sync.dma_start(
                out=x_sb[:].rearrange("p (t d) -> p t d", d=D),
                in_=x_d[:].rearrange("(t p) d -> p t d", p=P),
            )
            cb_sb = constp.tile([P, CBT * D], F32)
            nc.sync.dma_start(
                out=cb_sb[:].rearrange("p (t d) -> p t d", d=D),
                in_=cb_d[:].rearrange("(t p) d -> p t d", p=P),
            )

            # ---------- normalize codebook rows (fp32) -----------------
            nsq = constp.tile([P, CBT], F32)
            for j in range(CBT):
                sqt = workp.tile([P, D], F32, tag="sqt")
                nc.scalar.activation(
                    out=sqt[:],
                    in_=cb_sb[:, j * D : (j + 1) * D],
                    func=AF.Square,
                    accum_out=nsq[:, j : j + 1],
                )
            nrm = constp.tile([P, CBT], F32)
            nc.scalar.activation(out=nrm[:], in_=nsq[:], func=AF.Sqrt)
            rnrm = constp.tile([P, CBT], F32)
            nc.vector.reciprocal(out=rnrm[:], in_=nrm[:])
            cbn_sb = constp.tile([P, CBT * D], F32)
            for j in range(CBT):
                nc.vector.tensor_scalar_mul(
                    cbn_sb[:, j * D : (j + 1) * D],
                    cb_sb[:, j * D : (j + 1) * D],
                    rnrm[:, j : j + 1],
                )
            nc.sync.dma_start(
                out=cbn_d[:, 0:D].rearrange("(t p) d -> p t d", p=P),
                in_=cbn_sb[:].rearrange("p (t d) -> p t d", d=D),
            )
            nc.sync.dma_start(
                out=cbn_d[:, D : 2 * D].rearrange("(t p) d -> p t d", p=P),
                in_=cb_sb[:].rearrange("p (t d) -> p t d", d=D),
            )

            # ---------- operand prep + transposes ----------------------
            if mm_mode == "f16x2":
                # fp16 hi/lo splits, written straight into interleaved
                # (128,128)-per-tile pair buffers so the DMA xbar transpose
                # (needs full 128-wide tiles, 2-byte dtype) can build the
                # contraction-major layouts without touching PE/ACT:
                #   xhl_pair tile t cols = [xh | xl] -> T -> xhlT rows [xh^T; xl^T]
                #   xlh_pair tile t cols = [xl | xh] -> T -> xlhT rows [xl^T; xh^T]
                #   clh_pair tile j cols = [cl | ch] -> T -> clhT rows [cl^T; ch^T]
                xhl_pair = constp.tile([P, TILES * P], F16)
                xlh_pair = constp.tile([P, TILES * P], F16)
                clh_pair = constp.tile([P, CBT * P], F16)
                xp3 = xhl_pair[:].rearrange("p (t d) -> p t d", d=P)
                xq3 = xlh_pair[:].rearrange("p (t d) -> p t d", d=P)
                cp3 = clh_pair[:].rearrange("p (t d) -> p t d", d=P)
                xs3 = x_sb[:].rearrange("p (t d) -> p t d", d=D)
                cs3 = cbn_sb[:].rearrange("p (t d) -> p t d", d=D)
                nc.vector.tensor_copy(out=xp3[:, :, 0:D], in_=xs3)
                nc.vector.tensor_tensor(
                    out=xp3[:, :, D:P], in0=xs3, in1=xp3[:, :, 0:D], op=A.subtract
                )
                nc.vector.tensor_copy(out=xq3[:, :, D:P], in_=xp3[:, :, 0:D])
                nc.vector.tensor_copy(out=xq3[:, :, 0:D], in_=xp3[:, :, D:P])
                nc.scalar.copy(out=cp3[:, :, D:P], in_=cs3)
                nc.vector.tensor_tensor(
                    out=cp3[:, :, 0:D], in0=cs3, in1=cp3[:, :, D:P], op=A.subtract
                )
                xhlT = constp.tile([P, NTOK], F16)
                xlhT = constp.tile([P, NTOK], F16)
                clhT = constp.tile([P, V], F16)
                for j in range(CBT):
                    sl = slice(j * P, (j + 1) * P)
                    nc.sync.dma_start_transpose(
                        out=clhT[:, sl], in_=clh_pair[:, sl]
                    )
                for t in range(TILES):
                    sl = slice(t * P, (t + 1) * P)
                    nc.sync.dma_start_transpose(
                        out=xhlT[:, sl], in_=xhl_pair[:, sl]
                    )
                    nc.sync.dma_start_transpose(
                        out=xlhT[:, sl], in_=xlh_pair[:, sl]
                    )
                # hi-matmul operands at matching base partition 64
                xhT = xlhT[D:P]
                chT = clhT[D:P]
            else:
                ident32 = constp.tile([P, P], F32)
                make_identity(nc, ident32[:])
                cbT = constp.tile([D, V], F32)
                xT = constp.tile([D, NTOK], F32)
                with tc.tile_pool(name="psT", bufs=4, space="PSUM") as psT:
                    k = 0
                    for j in range(CBT):
                        pt = psT.tile([D, P], F32, tag="pt", name=f"pt_{k}")
                        nc.tensor.transpose(
                            out=pt[:], in_=cbn_sb[:, j * D : (j + 1) * D],
                            identity=ident32[:],
                        )
                        nc.scalar.copy(out=cbT[:, j * P : (j + 1) * P], in_=pt[:])
                        k += 1
                    for t in range(TILES):
                        pt = psT.tile([D, P], F32, tag="pt", name=f"pt_{k}")
                        nc.tensor.transpose(
                            out=pt[:], in_=x_sb[:, t * D : (t + 1) * D],
                            identity=ident32[:],
                        )
                        nc.scalar.copy(out=xT[:, t * P : (t + 1) * P], in_=pt[:])
                        k += 1

            # ---------- main loop --------------------------------------
            mi_all = constp.tile([P, TILES], U32)
            # per-candidate combined rows [cbn | cb], one 128-wide gather per
            # tile per candidate (multi-offset gathers are broken on HW)
            c1all = constp.tile([P, TILES * 2 * D], F32)
            c2all = constp.tile([P, TILES * 2 * D], F32)
            with tc.tile_pool(name="psC", bufs=4, space="PSUM") as psC:
                for t in range(TILES):
                    tsl = slice(t * P, (t + 1) * P)
                    sims_sb = workp.tile([P, V], F32, tag="sims", name=f"sims_{t}")
                    chunks = []
                    for c in range(V // 1024):
                        pt = psC.tile([P, 1024], F32, tag="pc", name=f"pc_{t}_{c}")
                        chunks.append(pt)
                        for h in range(2):
                            csl = slice((2 * c + h) * 512, (2 * c + h + 1) * 512)
                            if mm_mode == "f16x2":
                                nc.tensor.matmul(
                                    out=pt[:, h * 512 : (h + 1) * 512],
                                    lhsT=xhT[:, tsl], rhs=chT[:, csl],
                                    start=True, stop=False,
                                )
                            else:
                                nc.tensor.matmul(
                                    out=pt[:, h * 512 : (h + 1) * 512],
                                    lhsT=xT[:, tsl], rhs=cbT[:, csl],
                                    start=True, stop=True,
                                )
                    for c in range(V // 1024):
                        pt = chunks[c]
                        if mm_mode == "f16x2":
                            for h in range(2):
                                csl = slice((2 * c + h) * 512, (2 * c + h + 1) * 512)
                                nc.tensor.matmul(
                                    out=pt[:, h * 512 : (h + 1) * 512],
                                    lhsT=xhlT[:, tsl], rhs=clhT[:, csl],
                                    start=False, stop=True,
                                )
                        nc.scalar.copy(
                            out=sims_sb[:, c * 1024 : (c + 1) * 1024], in_=pt[:]
                        )

                    # fused fold + running max, one DVE pass (all-SBUF)
                    s1 = workp.tile([P, VH], F32, tag="s1", name=f"s1_{t}")
                    nc.vector.tensor_tensor_scan(
                        out=s1[:], data0=sims_sb[:, :VH], data1=sims_sb[:, VH:],
                        initial=NEG_INF, op0=A.max, op1=A.max,
                    )
                    gmax8 = smallp.tile([P, 8], F32, tag="gmax8", name=f"g8_{t}")
                    nc.vector.tensor_copy(
                        out=gmax8[:], in_=s1[:, VH - 1 : VH].to_broadcast([P, 8])
                    )
                    mi8 = smallp.tile([P, 8], U32, tag="mi8", name=f"mi8_{t}")
                    nc.vector.max_index(out=mi8[:], in_max=gmax8[:], in_values=s1[:])
                    nc.vector.tensor_copy(out=mi_all[:, t : t + 1], in_=mi8[:, 0:1])
                    fp2 = smallp.tile([P, 1], U32, tag="fp2", name=f"fp2_{t}")
                    nc.vector.tensor_scalar_add(fp2[:], mi8[:, 0:1], VH)
                    # overlap the candidate-row gathers with later tiles
                    nc.gpsimd.indirect_dma_start(
                        out=c1all[:, t * 2 * D : (t + 1) * 2 * D],
                        out_offset=None, in_=cbn_d[:],
                        in_offset=bass.IndirectOffsetOnAxis(ap=mi8[:, 0:1], axis=0),
                    )
                    nc.gpsimd.indirect_dma_start(
                        out=c2all[:, t * 2 * D : (t + 1) * 2 * D],
                        out_offset=None, in_=cbn_d[:],
                        in_offset=bass.IndirectOffsetOnAxis(ap=fp2[:], axis=0),
                    )

            # ---------- batched endgame --------------------------------
            # exact fp32 rescore d_i = x . cbn[cand_i]; winner by is_ge
            c1n = c1all[:].rearrange("p (t d) -> p t d", d=2 * D)[:, :, 0:D]
            c2n = c2all[:].rearrange("p (t d) -> p t d", d=2 * D)[:, :, 0:D]
            c1r = c1all[:].rearrange("p (t d) -> p t d", d=2 * D)[:, :, D : 2 * D]
            c2r = c2all[:].rearrange("p (t d) -> p t d", d=2 * D)[:, :, D : 2 * D]
            x3 = x_sb[:].rearrange("p (t d) -> p t d", d=D)
            tr1 = constp.tile([P, TILES * D], F32)
            tr2 = constp.tile([P, TILES * D], F32)
            d1 = constp.tile([P, TILES], F32)
            d2 = constp.tile([P, TILES], F32)
            nc.vector.tensor_tensor(
                out=tr1[:].rearrange("p (t d) -> p t d", d=D), in0=x3, in1=c1n,
                op=A.mult,
            )
            nc.vector.tensor_reduce(
                out=d1[:], in_=tr1[:].rearrange("p (t d) -> p t d", d=D),
                axis=mybir.AxisListType.X, op=A.add,
            )
            nc.vector.tensor_tensor(
                out=tr2[:].rearrange("p (t d) -> p t d", d=D), in0=x3, in1=c2n,
                op=A.mult,
            )
            nc.vector.tensor_reduce(
                out=d2[:], in_=tr2[:].rearrange("p (t d) -> p t d", d=D),
                axis=mybir.AxisListType.X, op=A.add,
            )
            mask = constp.tile([P, TILES], U32)
            nc.vector.tensor_tensor(out=mask[:], in0=d1[:], in1=d2[:], op=A.is_ge)
            # win = mask ? mi_all : mi_all + VH
            win = constp.tile([P, TILES], U32)
            nc.vector.tensor_scalar_add(win[:], mi_all[:], VH)
            nc.vector.copy_predicated(out=win[:], mask=mask[:], data=mi_all[:])
            idx_all = constp.tile([P, TILES], I32)
            nc.vector.tensor_copy(out=idx_all[:], in_=win[:])

            # z_q = mask ? raw(cand1) : raw(cand2)  (raw rows came with the gathers)
            mask64 = constp.tile([P, TILES * D], U32)
            nc.vector.tensor_copy(
                out=mask64[:].rearrange("p (t d) -> p t d", d=D),
                in_=mask[:].rearrange("p (t o) -> p t o", o=1).to_broadcast(
                    [P, TILES, D]
                ),
            )
            c1raw = constp.tile([P, TILES * D], F32)
            nc.vector.tensor_copy(
                out=c1raw[:].rearrange("p (t d) -> p t d", d=D), in_=c1r
            )
            zq_all = constp.tile([P, TILES * D], F32)
            nc.vector.tensor_copy(
                out=zq_all[:].rearrange("p (t d) -> p t d", d=D), in_=c2r
            )
            nc.vector.copy_predicated(
                out=zq_all[:], mask=mask64[:], data=c1raw[:]
            )
            nc.sync.dma_start(
                out=zq_d[:].rearrange("(t p) d -> p t d", p=P),
                in_=zq_all[:].rearrange("p (t d) -> p t d", d=D),
            )
            nc.sync.dma_start(
                out=idx_d[:].rearrange("(t p) o -> p t o", p=P),
                in_=idx_all[:].rearrange("p (t o) -> p t o", o=1),
            )

    nc.compile()
    return nc


_NC_CACHE = {}


def _get_nc(mm_mode=None):
    key = str(MM_MODE if mm_mode is None else mm_mode)
    if key not in _NC_CACHE:
        _NC_CACHE[key] = build_nc(mm_mode)
    return _NC_CACHE[key]


def kernel(x, codebook, gumbel_noise=None, _trace=False, _mm_mode=None):
    """Full-input entry point: x (8, 2048, 64) f32, codebook (4096, 64) f32.

    gumbel_noise is accepted for signature compatibility; it does not
    affect the forward value (term - stop_gradient(term) == 0).
    Returns (z_q_ste (8, 2048, 64) f32, indices (8, 2048) int32).
    """
    from concourse.bass_utils import run_bass_kernel_spmd

    x = np.ascontiguousarray(np.asarray(x, dtype=np.float32))
    codebook = np.ascontiguousarray(np.asarray(codebook, dtype=np.float32))
    shards = x.reshape(-1, D).reshape(N_CORES, NTOK, D)

    nc = _get_nc(_mm_mode)
    in_maps = [{"x": shards[i], "cb": codebook} for i in range(N_CORES)]
    kwargs = {}
    if _trace:
        import tempfile

        kwargs = {"trace": True, "tmpdir": tempfile.mkdtemp(prefix="vq_trace_")}
    res = run_bass_kernel_spmd(
        nc, in_maps, core_ids=list(range(N_CORES)), **kwargs
    )
    zq = np.concatenate([r["zq"] for r in res.results], axis=0)
    idx = np.concatenate([r["idx"] for r in res.results], axis=0)
    zq = zq.reshape(B, T, D).astype(np.float32, copy=False)
    idx = idx.reshape(B, T).astype(np.int32, copy=False)
    if _trace:
        kernel._last_results = res
    return zq, idx


if __name__ == "__main__":
    nc = build_nc()
    print("compiled OK")


# revision 23
# speedup vs baseline: 1.2284x; 1.2284x over previous
"""Trainium2 Bass kernel for nn_EpistemicQuantizer (vq_codebook).

Reference forward semantics:
    x_flat  = x.reshape(N, D)
    sims    = l2norm(x_flat) @ l2norm(codebook).T          # (N, V)
    indices = argmax_v sims                                # (N,)
    soft    = softmax((sims + gumbel)/tau)
    term    = soft @ sg(codebook)
    z_q     = codebook[indices] + term - sg(term)          # == codebook[indices] (+~1e-7 fp noise)
    out     = x + sg(z_q - x) + (z_q - sg(z_q))            # == z_q  (+~1e-7 fp noise)

The forward value is exactly (codebook[indices], indices) up to ~1e-7
relative fp noise: term - sg(term) and z_q - sg(z_q) cancel in the
forward pass, so the 256MB gumbel_noise tensor and the softmax never
affect the output.  Also argmax_v cos(x, c_v) == argmax_v (x . c_v /
||c_v||): per-token normalization of x is a positive per-row scale.

Device algorithm, data-parallel over tokens (2048 tokens/core x 8 cores):
  prep: normalize codebook rows (fp32), build fp16 hi/lo split of x and
        cbn, transpose to contraction-major layouts.
  sims via exact-enough fp16 decomposition (x = xh + xl, c = ch + cl):
        sims = xh@ch + [xh;xl]@[cl;ch]     (2 matmuls, error <= ~3e-6;
        measured 0 argmax flips vs fp64 on this problem's data, min
        top1-top2 gap is 9.8e-7 ... 1e-5 for the closest tokens)
  per 128-token tile:
    PE   : sims tile (128, 4096) fp32 in PSUM, 4 chunks of (128, 1024)
    ACT  : drain each PSUM chunk -> sims_sb in SBUF
    DVE  : tensor_tensor_scan(max, max) over (sims_sb half A, half B):
           one pass producing the running max of the pairwise fold; the
           last column is the global max, and max_index of it over the
           scan array is the first folded position p achieving it.
           Candidates v = p and v = p + 2048 are then fetched per tile
           with single-offset indirect-DMA gathers of combined
           [cbn | cb] rows (overlapped with later tiles).
  batched endgame (all 16 tiles at once on DVE):
    exact fp32 rescore d_i = x . cbn[cand_i] (min fold-partner gap here
    is 5.1e-3 vs ~2e-7 rescore noise), winner select by is_ge +
    copy_predicated; z_q taken from the raw halves of the gathered rows
    (no second gather); strided DMA writes z_q and int32 indices.
"""

import os
import sys

import numpy as np

for _p in ("/opt/trn_rl_repo", "/root/.axon_site/_ro/trn_rl_repo"):
    if os.path.isdir(_p) and _p not in sys.path:
        sys.path.insert(0, _p)

from concourse import bacc, bass, mybir  # noqa: E402
from concourse.masks import make_identity  # noqa: E402
from concourse.tile import TileContext  # noqa: E402

P = 128                    # partitions / tokens per tile
D = 64                     # token dim
V = 4096                   # codebook rows
N_CORES = 8
B, T = 8, 2048
NTOK = B * T // N_CORES    # tokens per core = 2048
TILES = NTOK // P          # 16
VH = V // 2                # folded width 2048
CBT = V // P               # 32 codebook tiles

F32 = mybir.dt.float32
F16 = mybir.dt.float16
I32 = mybir.dt.int32
U32 = mybir.dt.uint32
A = mybir.AluOpType
AF = mybir.ActivationFunctionType
NEG_INF = -3.0e38

MM_MODE = "f16x2"  # "f16x2" (2 fp16 matmuls/chunk) or "f32" (1 fp32 = 4 passes)


def build_nc(mm_mode=None):
    mm_mode = MM_MODE if mm_mode is None else mm_mode
    nc = bacc.Bacc("TRN2", target_bir_lowering=False)

    x_d = nc.dram_tensor("x", (NTOK, D), F32, kind="ExternalInput")
    cb_d = nc.dram_tensor("cb", (V, D), F32, kind="ExternalInput")
    zq_d = nc.dram_tensor("zq", (NTOK, D), F32, kind="ExternalOutput")
    idx_d = nc.dram_tensor("idx", (NTOK, 1), I32, kind="ExternalOutput")
    # combined scratch: row v = [cbn[v] (64) | cb[v] (64)] so one gather per
    # candidate fetches both the normalized row (rescore dot) and the raw row
    # (z_q output)
    cbn_d = nc.dram_tensor("cbn", (V, 2 * D), F32)

    with TileContext(nc) as tc:
        with (
            tc.tile_pool(name="const", bufs=1) as constp,
            tc.tile_pool(name="work", bufs=2) as workp,
            tc.tile_pool(name="small", bufs=3) as smallp,
        ):
            # ---------- loads ------------------------------------------
            # x_sb[p, t*D + d] = x[t*P + p, d]
            x_sb = constp.tile([P, TILES * D], F32)
            nc.sync.dma_start(
                out=x_sb[:].rearrange("p (t d) -> p t d", d=D),
                in_=x_d[:].rearrange("(t p) d -> p t d", p=P),
            )
            cb_sb = constp.tile([P, CBT * D], F32)
            nc.sync.dma_start(
                out=cb_sb[:].rearrange("p (t d) -> p t d", d=D),
                in_=cb_d[:].rearrange("(t p) d -> p t d", p=P),
            )

            # ---------- normalize codebook rows (fp32) -----------------
            nsq = constp.tile([P, CBT], F32)
            for j in range(CBT):
                sqt = workp.tile([P, D], F32, tag="sqt")
                nc.scalar.activation(
                    out=sqt[:],
                    in_=cb_sb[:, j * D : (j + 1) * D],
                    func=AF.Square,
                    accum_out=nsq[:, j : j + 1],
                )
            nrm = constp.tile([P, CBT], F32)
            nc.scalar.activation(out=nrm[:], in_=nsq[:], func=AF.Sqrt)
            rnrm = constp.tile([P, CBT], F32)
            nc.vector.reciprocal(out=rnrm[:], in_=nrm[:])
            cbn_sb = constp.tile([P, CBT * D], F32)
            for j in range(CBT):
                nc.vector.tensor_scalar_mul(
                    cbn_sb[:, j * D : (j + 1) * D],
                    cb_sb[:, j * D : (j + 1) * D],
                    rnrm[:, j : j + 1],
                )
            nc.sync.dma_start(
                out=cbn_d[:, 0:D].rearrange("(t p) d -> p t d", p=P),
                in_=cbn_sb[:].rearrange("p (t d) -> p t d", d=D),
            )
            nc.sync.dma_start(
                out=cbn_d[:, D : 2 * D].rearrange("(t p) d -> p t d", p=P),
                in_=cb_sb[:].rearrange("p (t d) -> p t d", d=D),
            )

            # ---------- operand prep + transposes ----------------------
            if mm_mode == "f16x2":
                ident16 = constp.tile([P, P], F16)
                make_identity(nc, ident16[:])
                # fp16 hi/lo splits
                xh_sb = constp.tile([P, TILES * D], F16)
                nc.vector.tensor_copy(out=xh_sb[:], in_=x_sb[:])
                xl_sb = constp.tile([P, TILES * D], F16)
                nc.vector.tensor_tensor(
                    out=xl_sb[:], in0=x_sb[:], in1=xh_sb[:], op=A.subtract
                )
                ch_sb = constp.tile([P, CBT * D], F16)
                nc.vector.tensor_copy(out=ch_sb[:], in_=cbn_sb[:])
                cl_sb = constp.tile([P, CBT * D], F16)
                nc.vector.tensor_tensor(
                    out=cl_sb[:], in0=cbn_sb[:], in1=ch_sb[:], op=A.subtract
                )
                # contraction-major layouts (PE needs equal base partitions,
                # so the hi-only operands get standalone 64-partition tensors):
                #   xhlT rows 0:64 = xh^T, 64:128 = xl^T    (128, NTOK)
                #   clhT rows 0:64 = cl^T, 64:128 = ch^T    (128, V)
                #   xhT  = xh^T (64, NTOK);  chT = ch^T (64, V)
                xhlT = constp.tile([P, NTOK], F16)
                clhT = constp.tile([P, V], F16)
                # full 128 contraction rows (rows 64:128 zero) so every
                # matmul has NumWeights==128 and gets the fast weight load
                xhT = constp.tile([P, NTOK], F16)
                chT = constp.tile([P, V], F16)
                nc.gpsimd.memset(xhT[D:P, :], 0.0)
                nc.gpsimd.memset(chT[D:P, :], 0.0)
                with tc.tile_pool(name="psT", bufs=4, space="PSUM") as psT:
                    def trans(src_ap, dsts, k):
                        pt = psT.tile([D, P], F16, tag="pt", name=f"pt_{k}")
                        nc.tensor.matmul(
                            out=pt[:], lhsT=src_ap, rhs=ident16[:],
                            is_transpose=True, start=True, stop=True,
                        )
                        # alternate engines for the PSUM->SBUF cast copies
                        for di, dst_ap in enumerate(dsts):
                            if (k + di) % 2 == 0:
                                nc.scalar.copy(out=dst_ap, in_=pt[:])
                            else:
                                nc.vector.tensor_copy(out=dst_ap, in_=pt[:])

                    k = 0
                    for j in range(CBT):
                        sl = slice(j * P, (j + 1) * P)
                        trans(cl_sb[:, j * D : (j + 1) * D],
                              [clhT[0:D, sl]], k); k += 1
                        trans(ch_sb[:, j * D : (j + 1) * D],
                              [clhT[D:P, sl], chT[0:D, sl]], k); k += 1
                    for t in range(TILES):
                        sl = slice(t * P, (t + 1) * P)
                        trans(xh_sb[:, t * D : (t + 1) * D],
                              [xhlT[0:D, sl], xhT[0:D, sl]], k); k += 1
                        trans(xl_sb[:, t * D : (t + 1) * D],
                              [xhlT[D:P, sl]], k); k += 1
            else:
                ident32 = constp.tile([P, P], F32)
                make_identity(nc, ident32[:])
                cbT = constp.tile([D, V], F32)
                xT = constp.tile([D, NTOK], F32)
                with tc.tile_pool(name="psT", bufs=4, space="PSUM") as psT:
                    k = 0
                    for j in range(CBT):
                        pt = psT.tile([D, P], F32, tag="pt", name=f"pt_{k}")
                        nc.tensor.transpose(
                            out=pt[:], in_=cbn_sb[:, j * D : (j + 1) * D],
                            identity=ident32[:],
                        )
                        nc.scalar.copy(out=cbT[:, j * P : (j + 1) * P], in_=pt[:])
                        k += 1
                    for t in range(TILES):
                        pt = psT.tile([D, P], F32, tag="pt", name=f"pt_{k}")
                        nc.tensor.transpose(
                            out=pt[:], in_=x_sb[:, t * D : (t + 1) * D],
                            identity=ident32[:],
                        )
                        nc.scalar.copy(out=xT[:, t * P : (t + 1) * P], in_=pt[:])
                        k += 1

            # ---------- main loop --------------------------------------
            mi_all = constp.tile([P, TILES], U32)
            # per-candidate combined rows [cbn | cb], one 128-wide gather per
            # tile per candidate (multi-offset gathers are broken on HW)
            c1all = constp.tile([P, TILES * 2 * D], F32)
            c2all = constp.tile([P, TILES * 2 * D], F32)
            with tc.tile_pool(name="psC", bufs=4, space="PSUM") as psC:
                for t in range(TILES):
                    tsl = slice(t * P, (t + 1) * P)
                    sims_sb = workp.tile([P, V], F32, tag="sims", name=f"sims_{t}")
                    chunks = []
                    for c in range(V // 1024):
                        pt = psC.tile([P, 1024], F32, tag="pc", name=f"pc_{t}_{c}")
                        chunks.append(pt)
                        for h in range(2):
                            csl = slice((2 * c + h) * 512, (2 * c + h + 1) * 512)
                            if mm_mode == "f16x2":
                                nc.tensor.matmul(
                                    out=pt[:, h * 512 : (h + 1) * 512],
                                    lhsT=xhT[:, tsl], rhs=chT[:, csl],
                                    start=True, stop=False,
                                )
                            else:
                                nc.tensor.matmul(
                                    out=pt[:, h * 512 : (h + 1) * 512],
                                    lhsT=xT[:, tsl], rhs=cbT[:, csl],
                                    start=True, stop=True,
                                )
                    for c in range(V // 1024):
                        pt = chunks[c]
                        if mm_mode == "f16x2":
                            for h in range(2):
                                csl = slice((2 * c + h) * 512, (2 * c + h + 1) * 512)
                                nc.tensor.matmul(
                                    out=pt[:, h * 512 : (h + 1) * 512],
                                    lhsT=xhlT[:, tsl], rhs=clhT[:, csl],
                                    start=False, stop=True,
                                )
                        nc.scalar.copy(
                            out=sims_sb[:, c * 1024 : (c + 1) * 1024], in_=pt[:]
                        )

                    # fused fold + running max, one DVE pass (all-SBUF)
                    s1 = workp.tile([P, VH], F32, tag="s1", name=f"s1_{t}")
                    nc.vector.tensor_tensor_scan(
                        out=s1[:], data0=sims_sb[:, :VH], data1=sims_sb[:, VH:],
                        initial=NEG_INF, op0=A.max, op1=A.max,
                    )
                    gmax8 = smallp.tile([P, 8], F32, tag="gmax8", name=f"g8_{t}")
                    nc.vector.tensor_copy(
                        out=gmax8[:], in_=s1[:, VH - 1 : VH].to_broadcast([P, 8])
                    )
                    mi8 = smallp.tile([P, 8], U32, tag="mi8", name=f"mi8_{t}")
                    nc.vector.max_index(out=mi8[:], in_max=gmax8[:], in_values=s1[:])
                    nc.vector.tensor_copy(out=mi_all[:, t : t + 1], in_=mi8[:, 0:1])
                    fp2 = smallp.tile([P, 1], U32, tag="fp2", name=f"fp2_{t}")
                    nc.vector.tensor_scalar_add(fp2[:], mi8[:, 0:1], VH)
                    # overlap the candidate-row gathers with later tiles
                    nc.gpsimd.indirect_dma_start(
                        out=c1all[:, t * 2 * D : (t + 1) * 2 * D],
                        out_offset=None, in_=cbn_d[:],
                        in_offset=bass.IndirectOffsetOnAxis(ap=mi8[:, 0:1], axis=0),
                    )
                    nc.gpsimd.indirect_dma_start(
                        out=c2all[:, t * 2 * D : (t + 1) * 2 * D],
                        out_offset=None, in_=cbn_d[:],
                        in_offset=bass.IndirectOffsetOnAxis(ap=fp2[:], axis=0),
                    )

            # ---------- batched endgame --------------------------------
            # exact fp32 rescore d_i = x . cbn[cand_i]; winner by is_ge
            c1n = c1all[:].rearrange("p (t d) -> p t d", d=2 * D)[:, :, 0:D]
            c2n = c2all[:].rearrange("p (t d) -> p t d", d=2 * D)[:, :, 0:D]
            c1r = c1all[:].rearrange("p (t d) -> p t d", d=2 * D)[:, :, D : 2 * D]
            c2r = c2all[:].rearrange("p (t d) -> p t d", d=2 * D)[:, :, D : 2 * D]
            x3 = x_sb[:].rearrange("p (t d) -> p t d", d=D)
            tr1 = constp.tile([P, TILES * D], F32)
            tr2 = constp.tile([P, TILES * D], F32)
            d1 = constp.tile([P, TILES], F32)
            d2 = constp.tile([P, TILES], F32)
            nc.vector.tensor_tensor(
                out=tr1[:].rearrange("p (t d) -> p t d", d=D), in0=x3, in1=c1n,
                op=A.mult,
            )
            nc.vector.tensor_reduce(
                out=d1[:], in_=tr1[:].rearrange("p (t d) -> p t d", d=D),
                axis=mybir.AxisListType.X, op=A.add,
            )
            nc.vector.tensor_tensor(
                out=tr2[:].rearrange("p (t d) -> p t d", d=D), in0=x3, in1=c2n,
                op=A.mult,
            )
            nc.vector.tensor_reduce(
                out=d2[:], in_=tr2[:].rearrange("p (t d) -> p t d", d=D),
                axis=mybir.AxisListType.X, op=A.add,
            )
            mask = constp.tile([P, TILES], U32)
            nc.vector.tensor_tensor(out=mask[:], in0=d1[:], in1=d2[:], op=A.is_ge)
            # win = mask ? mi_all : mi_all + VH
            win = constp.tile([P, TILES], U32)
            nc.vector.tensor_scalar_add(win[:], mi_all[:], VH)
            nc.vector.copy_predicated(out=win[:], mask=mask[:], data=mi_all[:])
            idx_all = constp.tile([P, TILES], I32)
            nc.vector.tensor_copy(out=idx_all[:], in_=win[:])

            # z_q = mask ? raw(cand1) : raw(cand2)  (raw rows came with the gathers)
            mask64 = constp.tile([P, TILES * D], U32)
            nc.vector.tensor_copy(
                out=mask64[:].rearrange("p (t d) -> p t d", d=D),
                in_=mask[:].rearrange("p (t o) -> p t o", o=1).to_broadcast(
                    [P, TILES, D]
                ),
            )
            c1raw = constp.tile([P, TILES * D], F32)
            nc.vector.tensor_copy(
                out=c1raw[:].rearrange("p (t d) -> p t d", d=D), in_=c1r
            )
            zq_all = constp.tile([P, TILES * D], F32)
            nc.vector.tensor_copy(
                out=zq_all[:].rearrange("p (t d) -> p t d", d=D), in_=c2r
            )
            nc.vector.copy_predicated(
                out=zq_all[:], mask=mask64[:], data=c1raw[:]
            )
            nc.sync.dma_start(
                out=zq_d[:].rearrange("(t p) d -> p t d", p=P),
                in_=zq_all[:].rearrange("p (t d) -> p t d", d=D),
            )
            nc.sync.dma_start(
                out=idx_d[:].rearrange("(t p) o -> p t o", p=P),
                in_=idx_all[:].rearrange("p (t o) -> p t o", o=1),
            )

    nc.compile()
    return nc


_NC_CACHE = {}


def _get_nc(mm_mode=None):
    key = str(MM_MODE if mm_mode is None else mm_mode)
    if key not in _NC_CACHE:
        _NC_CACHE[key] = build_nc(mm_mode)
    return _NC_CACHE[key]


def kernel(x, codebook, gumbel_noise=None, _trace=False, _mm_mode=None):
    """Full-input entry point: x (8, 2048, 64) f32, codebook (4096, 64) f32.

    gumbel_noise is accepted for signature compatibility; it does not
    affect the forward value (term - stop_gradient(term) == 0).
    Returns (z_q_ste (8, 2048, 64) f32, indices (8, 2048) int32).
    """
    from concourse.bass_utils import run_bass_kernel_spmd

    x = np.ascontiguousarray(np.asarray(x, dtype=np.float32))
    codebook = np.ascontiguousarray(np.asarray(codebook, dtype=np.float32))
    shards = x.reshape(-1, D).reshape(N_CORES, NTOK, D)

    nc = _get_nc(_mm_mode)
    in_maps = [{"x": shards[i], "cb": codebook} for i in range(N_CORES)]
    kwargs = {}
    if _trace:
        import tempfile

        kwargs = {"trace": True, "tmpdir": tempfile.mkdtemp(prefix="vq_trace_")}
    res = run_bass_kernel_spmd(
        nc, in_maps, core_ids=list(range(N_CORES)), **kwargs
    )
    zq = np.concatenate([r["zq"] for r in res.results], axis=0)
    idx = np.concatenate([r["idx"] for r in res.results], axis=0)
    zq = zq.reshape(B, T, D).astype(np.float32, copy=False)
    idx = idx.reshape(B, T).astype(np.int32, copy=False)
    if _trace:
        kernel._last_results = res
    return zq, idx


if __name__ == "__main__":
    nc = build_nc()
    print("compiled OK")


# revision 24
# speedup vs baseline: 1.2392x; 1.0088x over previous
"""Trainium2 Bass kernel for nn_EpistemicQuantizer (vq_codebook).

Reference forward semantics:
    x_flat  = x.reshape(N, D)
    sims    = l2norm(x_flat) @ l2norm(codebook).T          # (N, V)
    indices = argmax_v sims                                # (N,)
    soft    = softmax((sims + gumbel)/tau)
    term    = soft @ sg(codebook)
    z_q     = codebook[indices] + term - sg(term)          # == codebook[indices] (+~1e-7 fp noise)
    out     = x + sg(z_q - x) + (z_q - sg(z_q))            # == z_q  (+~1e-7 fp noise)

The forward value is exactly (codebook[indices], indices) up to ~1e-7
relative fp noise: term - sg(term) and z_q - sg(z_q) cancel in the
forward pass, so the 256MB gumbel_noise tensor and the softmax never
affect the output.  Also argmax_v cos(x, c_v) == argmax_v (x . c_v /
||c_v||): per-token normalization of x is a positive per-row scale.

Device algorithm, data-parallel over tokens (2048 tokens/core x 8 cores):
  prep: normalize codebook rows (fp32), build fp16 hi/lo split of x and
        cbn, transpose to contraction-major layouts.
  sims via exact-enough fp16 decomposition (x = xh + xl, c = ch + cl):
        sims = xh@ch + [xh;xl]@[cl;ch]     (2 matmuls, error <= ~3e-6;
        measured 0 argmax flips vs fp64 on this problem's data, min
        top1-top2 gap is 9.8e-7 ... 1e-5 for the closest tokens)
  per 128-token tile:
    PE   : sims tile (128, 4096) fp32 in PSUM, 4 chunks of (128, 1024)
    ACT  : drain each PSUM chunk -> sims_sb in SBUF
    DVE  : tensor_tensor_scan(max, max) over (sims_sb half A, half B):
           one pass producing the running max of the pairwise fold; the
           last column is the global max, and max_index of it over the
           scan array is the first folded position p achieving it.
           Candidates v = p and v = p + 2048 are then fetched per tile
           with single-offset indirect-DMA gathers of combined
           [cbn | cb] rows (overlapped with later tiles).
  batched endgame (all 16 tiles at once on DVE):
    exact fp32 rescore d_i = x . cbn[cand_i] (min fold-partner gap here
    is 5.1e-3 vs ~2e-7 rescore noise), winner select by is_ge +
    copy_predicated; z_q taken from the raw halves of the gathered rows
    (no second gather); strided DMA writes z_q and int32 indices.
"""

import os
import sys

import numpy as np

for _p in ("/opt/trn_rl_repo", "/root/.axon_site/_ro/trn_rl_repo"):
    if os.path.isdir(_p) and _p not in sys.path:
        sys.path.insert(0, _p)

from concourse import bacc, bass, mybir  # noqa: E402
from concourse.masks import make_identity  # noqa: E402
from concourse.tile import TileContext  # noqa: E402

P = 128                    # partitions / tokens per tile
D = 64                     # token dim
V = 4096                   # codebook rows
N_CORES = 8
B, T = 8, 2048
NTOK = B * T // N_CORES    # tokens per core = 2048
TILES = NTOK // P          # 16
VH = V // 2                # folded width 2048
CBT = V // P               # 32 codebook tiles

F32 = mybir.dt.float32
F16 = mybir.dt.float16
I32 = mybir.dt.int32
U32 = mybir.dt.uint32
A = mybir.AluOpType
AF = mybir.ActivationFunctionType
NEG_INF = -3.0e38

MM_MODE = "f16x2"  # "f16x2" (2 fp16 matmuls/chunk) or "f32" (1 fp32 = 4 passes)


def build_nc(mm_mode=None):
    mm_mode = MM_MODE if mm_mode is None else mm_mode
    nc = bacc.Bacc("TRN2", target_bir_lowering=False)

    x_d = nc.dram_tensor("x", (NTOK, D), F32, kind="ExternalInput")
    cb_d = nc.dram_tensor("cb", (V, D), F32, kind="ExternalInput")
    zq_d = nc.dram_tensor("zq", (NTOK, D), F32, kind="ExternalOutput")
    idx_d = nc.dram_tensor("idx", (NTOK, 1), I32, kind="ExternalOutput")
    # combined scratch: row v = [cbn[v] (64) | cb[v] (64)] so one gather per
    # candidate fetches both the normalized row (rescore dot) and the raw row
    # (z_q output)
    cbn_d = nc.dram_tensor("cbn", (V, 2 * D), F32)

    with TileContext(nc) as tc:
        with (
            tc.tile_pool(name="const", bufs=1) as constp,
            tc.tile_pool(name="work", bufs=2) as workp,
            tc.tile_pool(name="small", bufs=3) as smallp,
        ):
            # ---------- loads ------------------------------------------
            # x_sb[p, t*D + d] = x[t*P + p, d]
            x_sb = constp.tile([P, TILES * D], F32)
            nc.sync.dma_start(
                out=x_sb[:].rearrange("p (t d) -> p t d", d=D),
                in_=x_d[:].rearrange("(t p) d -> p t d", p=P),
            )
            cb_sb = constp.tile([P, CBT * D], F32)
            nc.sync.dma_start(
                out=cb_sb[:].rearrange("p (t d) -> p t d", d=D),
                in_=cb_d[:].rearrange("(t p) d -> p t d", p=P),
            )

            # ---------- normalize codebook rows (fp32) -----------------
            nsq = constp.tile([P, CBT], F32)
            for j in range(CBT):
                sqt = workp.tile([P, D], F32, tag="sqt")
                nc.scalar.activation(
                    out=sqt[:],
                    in_=cb_sb[:, j * D : (j + 1) * D],
                    func=AF.Square,
                    accum_out=nsq[:, j : j + 1],
                )
            nrm = constp.tile([P, CBT], F32)
            nc.scalar.activation(out=nrm[:], in_=nsq[:], func=AF.Sqrt)
            rnrm = constp.tile([P, CBT], F32)
            nc.vector.reciprocal(out=rnrm[:], in_=nrm[:])
            cbn_sb = constp.tile([P, CBT * D], F32)
            for j in range(CBT):
                nc.vector.tensor_scalar_mul(
                    cbn_sb[:, j * D : (j + 1) * D],
                    cb_sb[:, j * D : (j + 1) * D],
                    rnrm[:, j : j + 1],
                )
            nc.sync.dma_start(
                out=cbn_d[:, 0:D].rearrange("(t p) d -> p t d", p=P),
                in_=cbn_sb[:].rearrange("p (t d) -> p t d", d=D),
            )
            nc.sync.dma_start(
                out=cbn_d[:, D : 2 * D].rearrange("(t p) d -> p t d", p=P),
                in_=cb_sb[:].rearrange("p (t d) -> p t d", d=D),
            )

            # ---------- operand prep + transposes ----------------------
            if mm_mode == "f16x2":
                ident16 = constp.tile([P, P], F16)
                make_identity(nc, ident16[:])
                # fp16 hi/lo splits
                xh_sb = constp.tile([P, TILES * D], F16)
                nc.vector.tensor_copy(out=xh_sb[:], in_=x_sb[:])
                xl_sb = constp.tile([P, TILES * D], F16)
                nc.vector.tensor_tensor(
                    out=xl_sb[:], in0=x_sb[:], in1=xh_sb[:], op=A.subtract
                )
                ch_sb = constp.tile([P, CBT * D], F16)
                nc.vector.tensor_copy(out=ch_sb[:], in_=cbn_sb[:])
                cl_sb = constp.tile([P, CBT * D], F16)
                nc.vector.tensor_tensor(
                    out=cl_sb[:], in0=cbn_sb[:], in1=ch_sb[:], op=A.subtract
                )
                # contraction-major layouts (PE needs equal base partitions,
                # so the hi-only operands get standalone 64-partition tensors):
                #   xhlT rows 0:64 = xh^T, 64:128 = xl^T    (128, NTOK)
                #   clhT rows 0:64 = cl^T, 64:128 = ch^T    (128, V)
                #   xhT  = xh^T (64, NTOK);  chT = ch^T (64, V)
                xhlT = constp.tile([P, NTOK], F16)
                clhT = constp.tile([P, V], F16)
                # full 128 contraction rows (rows 64:128 zero) so every
                # matmul has NumWeights==128 and gets the fast weight load
                xhT = constp.tile([P, NTOK], F16)
                chT = constp.tile([P, V], F16)
                nc.gpsimd.memset(xhT[D:P, :], 0.0)
                nc.gpsimd.memset(chT[D:P, :], 0.0)
                with tc.tile_pool(name="psT", bufs=4, space="PSUM") as psT:
                    def trans(src_ap, dsts, k):
                        pt = psT.tile([D, P], F16, tag="pt", name=f"pt_{k}")
                        nc.tensor.matmul(
                            out=pt[:], lhsT=src_ap, rhs=ident16[:],
                            is_transpose=True, start=True, stop=True,
                        )
                        # alternate engines for the PSUM->SBUF cast copies
                        for di, dst_ap in enumerate(dsts):
                            if (k + di) % 2 == 0:
                                nc.scalar.copy(out=dst_ap, in_=pt[:])
                            else:
                                nc.vector.tensor_copy(out=dst_ap, in_=pt[:])

                    k = 0
                    for j in range(CBT):
                        sl = slice(j * P, (j + 1) * P)
                        trans(cl_sb[:, j * D : (j + 1) * D],
                              [clhT[0:D, sl]], k); k += 1
                        trans(ch_sb[:, j * D : (j + 1) * D],
                              [clhT[D:P, sl], chT[0:D, sl]], k); k += 1
                    for t in range(TILES):
                        sl = slice(t * P, (t + 1) * P)
                        trans(xh_sb[:, t * D : (t + 1) * D],
                              [xhlT[0:D, sl], xhT[0:D, sl]], k); k += 1
                        trans(xl_sb[:, t * D : (t + 1) * D],
                              [xhlT[D:P, sl]], k); k += 1
            else:
                ident32 = constp.tile([P, P], F32)
                make_identity(nc, ident32[:])
                cbT = constp.tile([D, V], F32)
                xT = constp.tile([D, NTOK], F32)
                with tc.tile_pool(name="psT", bufs=4, space="PSUM") as psT:
                    k = 0
                    for j in range(CBT):
                        pt = psT.tile([D, P], F32, tag="pt", name=f"pt_{k}")
                        nc.tensor.transpose(
                            out=pt[:], in_=cbn_sb[:, j * D : (j + 1) * D],
                            identity=ident32[:],
                        )
                        nc.scalar.copy(out=cbT[:, j * P : (j + 1) * P], in_=pt[:])
                        k += 1
                    for t in range(TILES):
                        pt = psT.tile([D, P], F32, tag="pt", name=f"pt_{k}")
                        nc.tensor.transpose(
                            out=pt[:], in_=x_sb[:, t * D : (t + 1) * D],
                            identity=ident32[:],
                        )
                        nc.scalar.copy(out=xT[:, t * P : (t + 1) * P], in_=pt[:])
                        k += 1

            # ---------- main loop --------------------------------------
            mi_all = constp.tile([P, TILES], U32)
            # per-candidate combined rows [cbn | cb], one 128-wide gather per
            # tile per candidate (multi-offset gathers are broken on HW)
            c1all = constp.tile([P, TILES * 2 * D], F32)
            c2all = constp.tile([P, TILES * 2 * D], F32)
            with tc.tile_pool(name="psC", bufs=4, space="PSUM") as psC:
                for t in range(TILES):
                    tsl = slice(t * P, (t + 1) * P)
                    sims_sb = workp.tile([P, V], F32, tag="sims", name=f"sims_{t}", bufs=3)
                    chunks = []
                    for c in range(V // 1024):
                        pt = psC.tile([P, 1024], F32, tag="pc", name=f"pc_{t}_{c}")
                        chunks.append(pt)
                        for h in range(2):
                            csl = slice((2 * c + h) * 512, (2 * c + h + 1) * 512)
                            if mm_mode == "f16x2":
                                nc.tensor.matmul(
                                    out=pt[:, h * 512 : (h + 1) * 512],
                                    lhsT=xhT[:, tsl], rhs=chT[:, csl],
                                    start=True, stop=False,
                                )
                            else:
                                nc.tensor.matmul(
                                    out=pt[:, h * 512 : (h + 1) * 512],
                                    lhsT=xT[:, tsl], rhs=cbT[:, csl],
                                    start=True, stop=True,
                                )
                    for c in range(V // 1024):
                        pt = chunks[c]
                        if mm_mode == "f16x2":
                            for h in range(2):
                                csl = slice((2 * c + h) * 512, (2 * c + h + 1) * 512)
                                nc.tensor.matmul(
                                    out=pt[:, h * 512 : (h + 1) * 512],
                                    lhsT=xhlT[:, tsl], rhs=clhT[:, csl],
                                    start=False, stop=True,
                                )
                        nc.scalar.copy(
                            out=sims_sb[:, c * 1024 : (c + 1) * 1024], in_=pt[:]
                        )

                    # fused fold + running max, one DVE pass (all-SBUF)
                    s1 = workp.tile([P, VH], F32, tag="s1", name=f"s1_{t}")
                    nc.vector.tensor_tensor_scan(
                        out=s1[:], data0=sims_sb[:, :VH], data1=sims_sb[:, VH:],
                        initial=NEG_INF, op0=A.max, op1=A.max,
                    )
                    gmax8 = smallp.tile([P, 8], F32, tag="gmax8", name=f"g8_{t}")
                    nc.vector.tensor_copy(
                        out=gmax8[:], in_=s1[:, VH - 1 : VH].to_broadcast([P, 8])
                    )
                    mi8 = smallp.tile([P, 8], U32, tag="mi8", name=f"mi8_{t}")
                    nc.vector.max_index(out=mi8[:], in_max=gmax8[:], in_values=s1[:])
                    nc.vector.tensor_copy(out=mi_all[:, t : t + 1], in_=mi8[:, 0:1])
                    fp2 = smallp.tile([P, 1], U32, tag="fp2", name=f"fp2_{t}")
                    nc.vector.tensor_scalar_add(fp2[:], mi8[:, 0:1], VH)
                    # overlap the candidate-row gathers with later tiles
                    nc.gpsimd.indirect_dma_start(
                        out=c1all[:, t * 2 * D : (t + 1) * 2 * D],
                        out_offset=None, in_=cbn_d[:],
                        in_offset=bass.IndirectOffsetOnAxis(ap=mi8[:, 0:1], axis=0),
                    )
                    nc.gpsimd.indirect_dma_start(
                        out=c2all[:, t * 2 * D : (t + 1) * 2 * D],
                        out_offset=None, in_=cbn_d[:],
                        in_offset=bass.IndirectOffsetOnAxis(ap=fp2[:], axis=0),
                    )

            # ---------- batched endgame --------------------------------
            # exact fp32 rescore d_i = x . cbn[cand_i]; winner by is_ge
            c1n = c1all[:].rearrange("p (t d) -> p t d", d=2 * D)[:, :, 0:D]
            c2n = c2all[:].rearrange("p (t d) -> p t d", d=2 * D)[:, :, 0:D]
            c1r = c1all[:].rearrange("p (t d) -> p t d", d=2 * D)[:, :, D : 2 * D]
            c2r = c2all[:].rearrange("p (t d) -> p t d", d=2 * D)[:, :, D : 2 * D]
            x3 = x_sb[:].rearrange("p (t d) -> p t d", d=D)
            tr1 = constp.tile([P, TILES * D], F32)
            tr2 = constp.tile([P, TILES * D], F32)
            d1 = constp.tile([P, TILES], F32)
            d2 = constp.tile([P, TILES], F32)
            nc.vector.tensor_tensor(
                out=tr1[:].rearrange("p (t d) -> p t d", d=D), in0=x3, in1=c1n,
                op=A.mult,
            )
            nc.vector.tensor_reduce(
                out=d1[:], in_=tr1[:].rearrange("p (t d) -> p t d", d=D),
                axis=mybir.AxisListType.X, op=A.add,
            )
            nc.vector.tensor_tensor(
                out=tr2[:].rearrange("p (t d) -> p t d", d=D), in0=x3, in1=c2n,
                op=A.mult,
            )
            nc.vector.tensor_reduce(
                out=d2[:], in_=tr2[:].rearrange("p (t d) -> p t d", d=D),
                axis=mybir.AxisListType.X, op=A.add,
            )
            mask = constp.tile([P, TILES], U32)
            nc.vector.tensor_tensor(out=mask[:], in0=d1[:], in1=d2[:], op=A.is_ge)
            # win = mask ? mi_all : mi_all + VH
            win = constp.tile([P, TILES], U32)
            nc.vector.tensor_scalar_add(win[:], mi_all[:], VH)
            nc.vector.copy_predicated(out=win[:], mask=mask[:], data=mi_all[:])
            idx_all = constp.tile([P, TILES], I32)
            nc.vector.tensor_copy(out=idx_all[:], in_=win[:])

            # z_q = mask ? raw(cand1) : raw(cand2)  (raw rows came with the gathers)
            mask64 = constp.tile([P, TILES * D], U32)
            nc.vector.tensor_copy(
                out=mask64[:].rearrange("p (t d) -> p t d", d=D),
                in_=mask[:].rearrange("p (t o) -> p t o", o=1).to_broadcast(
                    [P, TILES, D]
                ),
            )
            c1raw = constp.tile([P, TILES * D], F32)
            nc.vector.tensor_copy(
                out=c1raw[:].rearrange("p (t d) -> p t d", d=D), in_=c1r
            )
            zq_all = constp.tile([P, TILES * D], F32)
            nc.vector.tensor_copy(
                out=zq_all[:].rearrange("p (t d) -> p t d", d=D), in_=c2r
            )
            nc.vector.copy_predicated(
                out=zq_all[:], mask=mask64[:], data=c1raw[:]
            )
            nc.sync.dma_start(
                out=zq_d[:].rearrange("(t p) d -> p t d", p=P),
                in_=zq_all[:].rearrange("p (t d) -> p t d", d=D),
            )
            nc.sync.dma_start(
                out=idx_d[:].rearrange("(t p) o -> p t o", p=P),
                in_=idx_all[:].rearrange("p (t o) -> p t o", o=1),
            )

    nc.compile()
    return nc


_NC_CACHE = {}


def _get_nc(mm_mode=None):
    key = str(MM_MODE if mm_mode is None else mm_mode)
    if key not in _NC_CACHE:
        _NC_CACHE[key] = build_nc(mm_mode)
    return _NC_CACHE[key]


def kernel(x, codebook, gumbel_noise=None, _trace=False, _mm_mode=None):
    """Full-input entry point: x (8, 2048, 64) f32, codebook (4096, 64) f32.

    gumbel_noise is accepted for signature compatibility; it does not
    affect the forward value (term - stop_gradient(term) == 0).
    Returns (z_q_ste (8, 2048, 64) f32, indices (8, 2048) int32).
    """
    from concourse.bass_utils import run_bass_kernel_spmd

    x = np.ascontiguousarray(np.asarray(x, dtype=np.float32))
    codebook = np.ascontiguousarray(np.asarray(codebook, dtype=np.float32))
    shards = x.reshape(-1, D).reshape(N_CORES, NTOK, D)

    nc = _get_nc(_mm_mode)
    in_maps = [{"x": shards[i], "cb": codebook} for i in range(N_CORES)]
    kwargs = {}
    if _trace:
        import tempfile

        kwargs = {"trace": True, "tmpdir": tempfile.mkdtemp(prefix="vq_trace_")}
    res = run_bass_kernel_spmd(
        nc, in_maps, core_ids=list(range(N_CORES)), **kwargs
    )
    zq = np.concatenate([r["zq"] for r in res.results], axis=0)
    idx = np.concatenate([r["idx"] for r in res.results], axis=0)
    zq = zq.reshape(B, T, D).astype(np.float32, copy=False)
    idx = idx.reshape(B, T).astype(np.int32, copy=False)
    if _trace:
        kernel._last_results = res
    return zq, idx


if __name__ == "__main__":
    nc = build_nc()
    print("compiled OK")
